# revision 20
# baseline (speedup 1.0000x reference)
"""AssistedExcitation distributed Bass kernel for 8 TRN2 NeuronCores.

Reference computation (per batch b):
    mask[h,w]  = union over 32 boxes of axis-aligned rectangles (rasterized
                 from normalized xywh boxes, trunc + clamp semantics)
    att        = 5x5 conv of reflect-padded mask with the given kernel
    out        = x + att * x        (att broadcast over 256 channels)

Sharding: pure data parallel — batch 16 is split 2-per-core across 8 cores.
No collectives needed.

Per-core algorithm (all bulk work on-device):
  * Box rasterization is a matmul: 0/1 interval-indicator rows
    Cm[n,pw] (cols) and Rv[n,ph] (rows x validity), evaluated at
    reflect-mapped padded coordinates m[p]=min(|p-2|,158-(p-2)), give
    PT[pw,ph] = #boxes covering the padded pixel via lhsT=Cm, rhs=Rv;
    binarize (>0) yields the *reflect-padded transposed* mask in one
    shot.  Cm/Rv are computed host-side in exact f32/trunc/clamp
    reference arithmetic and shipped inside the merged const tensor
    (64x84 f16 each) — the device attention path starts at the PT
    matmul the moment the consts land.
  * The 5x5 conv is 5 PSUM-accumulated matmuls with banded matrices
    Kc_i[pw,w] = k[i, pw-w]:  att[h,w] = sum_i sum_pw PT[pw,h+i]*Kc_i[pw,w].
    Kc (a pure repacking of the 25 kernel weights) and the broadcast
    ones-vector are precomputed host-side in the same const tensor.
  * (1+att) is broadcast across the 128 partitions with K=1 fp16 matmuls
    (lhsT = ones[1,128], rhs = fp16 flattened (1+att) row), evicted to
    SBUF f16, then out = x16 * att_bc on the VectorEngine (f32 result),
    streamed in [128, 1600] chunks.

Scheduling notes (the kernel is DMA-roofline-bound: ~26.4 MB of HBM
traffic per core against the ~435 GB/s 16-SDMA-engine pool; measured
~411 GB/s average with a zero-idle in-ring, within ~1% of the
achievable floor):
  * Two bulk DMA rings: the x in-stream rides the gpsimd SWDGE ring
    (casting f32 -> f16 inline), and the out-stream rides the sync
    HWDGE ring behind the tiny merged const DMA and the two att1->flat
    flatten DMAs.  Rings drain concurrently (packet round-robin at the
    SDMA engines), so the in-flood needs no trigger-order pins and
    never queues behind an out-trigger's mult semaphore.
  * With the in-stream off the HWDGE lanes entirely, the flat/const
    completions are observed on nearly-empty DMAHW lanes -- promptly,
    with no tick-order pins needed.
  * All 16 in-chunks are prefetched into 16 fp16 pool bufs up-front, so
    the in-ring streams unconditionally, never blocked on pool-slot
    reuse: during the attention-path latency the in-ring alone consumes
    the full SDMA bandwidth, making the out-start latency nearly free.
  * Chunks are k-major per batch and each batch's mults are emitted
    right after the broadcast eviction that completes their att_bc
    column range, so out0 fires ~5us after the flat lands rather than
    after the whole broadcast.
  * fp16 x-stream rel-err ~2e-4, far inside the 2e-2 gate; it halves
    SBUF footprint (enables the 16-deep in pool) and SBUF-side DMA
    writes.
"""

import numpy as np

import concourse.bass as bass
import concourse.tile as tile
from concourse import bacc, mybir
from concourse.tile_rust import add_dep_helper
from concourse.bass_utils import run_bass_kernel_spmd

F32 = mybir.dt.float32
F16 = mybir.dt.float16
ALU = mybir.AluOpType
ACT = mybir.ActivationFunctionType

N_CORES = 8
B, C, H, W, NBOX = 16, 256, 80, 80, 32
B_LOC = B // N_CORES          # 2 batches per core
HW = H * W                    # 6400
PAD = 84                      # 80 + 2*2 reflect pad
KS = 5
CH = 1600                     # free-dim chunk of the x stream
N_CHUNK = HW // CH            # 4
BC_CH = 512                   # psum bank width for the broadcast matmul

NB2 = B_LOC * NBOX            # 64 boxes across the two local batches
CST_COLS = KS * W + 128 + 2 * PAD   # kc | ones | cm | rv


def _build_nc():
    nc = bacc.Bacc(None, target_bir_lowering=False)

    x_d = nc.declare_dram_parameter("x", [B_LOC, C, H, W], F32, isOutput=False)
    nc.declare_dram_parameter("boxes", [B_LOC, NBOX, 4], F32, isOutput=False)
    nc.declare_dram_parameter("kernel", [1, 1, KS, KS], F32, isOutput=False)
    cst_d = nc.declare_dram_parameter("cst", [PAD, CST_COLS], F16, isOutput=False)
    out_d = nc.declare_dram_parameter("out", [B_LOC, C, H, W], F32, isOutput=True)

    xr = x_d.rearrange("b c h w -> b c (h w)")
    outr = out_d.rearrange("b c h w -> b c (h w)")

    with tile.TileContext(nc) as tc:
        with (
            tc.tile_pool(name="const", bufs=1) as cp,
            tc.tile_pool(name="batch", bufs=2) as bp,
            tc.tile_pool(name="attbc", bufs=2) as ap_,
            tc.tile_pool(name="xin", bufs=16) as xp,
            tc.tile_pool(name="xout", bufs=8) as op_,
            tc.tile_pool(name="ps_small", bufs=2, space=bass.MemorySpace.PSUM) as psm,
            tc.tile_pool(name="ps_bc", bufs=4, space=bass.MemorySpace.PSUM) as pbc,
        ):
            # Trigger chains: pin the issue order within the sync ring
            # (cst, flats, out0..out15) and the gpsimd ring (in0..in15).
            sync_chain = []

            def _chain_sync(bi):
                if sync_chain:
                    add_dep_helper(bi.ins, sync_chain[-1].ins, sync=False,
                                   reason="pin sync trigger order")
                sync_chain.append(bi)

            in_chain = []

            def _chain_in(bi):
                if in_chain:
                    add_dep_helper(bi.ins, in_chain[-1].ins, sync=False,
                                   reason="pin gpsimd in-trigger order")
                in_chain.append(bi)

            # Merged const DMA: first on the sync ring. Contents: banded conv
            # matrices, ones row, host-precomputed box indicator rows.
            cst = cp.tile([PAD, CST_COLS], F16)
            _chain_sync(nc.sync.dma_start(cst[:], cst_d[:]))
            kc = cst[:, 0 : KS * W]
            ones16 = cst[0:1, KS * W : KS * W + 128]
            cm = cst[0:NB2, KS * W + 128 : KS * W + 128 + PAD]
            rv = cst[0:NB2, KS * W + 128 + PAD : CST_COLS]

            # ------------- per-batch attention path (through flat) -------------
            # Both batches' rasterize+conv+flatten run first; the flats ride
            # the sync ring (empty until out0, so they drain immediately).
            flats = []
            for b in range(B_LOC):
                # rasterize: PT[pw, ph] = #boxes covering the (padded) pixel
                pt_ps = psm.tile([PAD, PAD], F32, tag="pt_ps")
                nc.tensor.matmul(
                    pt_ps[:],
                    cm[b * NBOX : (b + 1) * NBOX, :],
                    rv[b * NBOX : (b + 1) * NBOX, :],
                    start=True, stop=True,
                )
                ptm = bp.tile([PAD, PAD], F16)
                nc.vector.tensor_scalar(ptm[:], pt_ps[:], 0.5, None, op0=ALU.is_ge)

                # 5x5 conv: 5 accumulated matmuls
                att_ps = psm.tile([H, W], F32, tag="att_ps")
                for i in range(KS):
                    nc.tensor.matmul(
                        att_ps[:],
                        ptm[:, i : i + H],
                        kc[:, i * W : (i + 1) * W],
                        start=(i == 0),
                        stop=(i == KS - 1),
                    )
                # (1 + att), cast to fp16 for the cheap broadcast matmul
                att1 = bp.tile([H, W], F16)
                nc.scalar.activation(att1[:], att_ps[:], ACT.Copy, bias=1.0)
                flat = bp.tile([1, HW], F16)
                _chain_sync(nc.sync.dma_start(flat[:], att1[:]))
                flats.append(flat)

            # ---------------- main stream: out = x * (1 + att) ----------------
            # Chunks are k-major per batch so consecutive mults need
            # consecutive att_bc column ranges.  Per batch the broadcast
            # evictions (alternating vector/scalar) are interleaved with the
            # mults: mult(k) is emitted right after the eviction that
            # completes its [k*CH, (k+1)*CH) range, so out0 fires ~5us after
            # the flat lands instead of after the whole broadcast.  All 16
            # in-loads are prefetched up-front into 16 pool bufs (fp16 tiles)
            # so the in-ring streams unconditionally, never slot-blocked.
            chunks = [
                (b, chalf * 128, k * CH)
                for b in range(B_LOC)
                for k in range(N_CHUNK)
                for chalf in range(C // 128)
            ]
            xts = {}

            def _load(i):
                b, c0, o0 = chunks[i]
                xt = xp.tile([128, CH], F16, name=f"xt{i}", tag="xt")
                # SWDGE in-DMA with inline f32 -> f16 downcast
                bi = nc.gpsimd.dma_start(xt[:], xr[b, c0 : c0 + 128, o0 : o0 + CH])
                _chain_in(bi)
                xts[i] = xt

            for i in range(len(chunks)):
                _load(i)

            n_per_b = len(chunks) // B_LOC
            n_ev = (HW + BC_CH - 1) // BC_CH          # 13 evictions per batch

            def _mult_out(i):
                b, c0, o0 = chunks[i]
                xt = xts.pop(i)
                ot = op_.tile([128, CH], F32, name=f"ot{i}", tag="ot")
                nc.vector.tensor_mul(ot[:], xt[:], att_bcs[b][:, o0 : o0 + CH])
                _chain_sync(
                    nc.sync.dma_start(outr[b, c0 : c0 + 128, o0 : o0 + CH], ot[:])
                )

            att_bcs = []
            for b in range(B_LOC):
                att_bc = ap_.tile([128, HW], F16, tag="att_bc")
                att_bcs.append(att_bc)
                k_next = 0
                for ci in range(n_ev):
                    off = ci * BC_CH
                    cw = min(BC_CH, HW - off)
                    bc_ps = pbc.tile([128, BC_CH], F32, tag="bc_ps")
                    nc.tensor.matmul(
                        bc_ps[:, 0:cw], ones16[:], flats[b][:, off : off + cw],
                        start=True, stop=True,
                    )
                    if ci % 2 == 1:
                        nc.vector.tensor_copy(att_bc[:, off : off + cw], bc_ps[:, 0:cw])
                    else:
                        nc.scalar.copy(att_bc[:, off : off + cw], bc_ps[:, 0:cw])
                    # emit the mults whose column range is now fully evicted
                    while k_next < N_CHUNK and (k_next + 1) * CH <= off + cw:
                        for chalf in range(C // 128):
                            _mult_out(b * n_per_b + k_next * (C // 128) + chalf)
                        k_next += 1

    if not nc.is_finalized():
        nc.finalize()
    return nc


def _host_consts(ker: np.ndarray, boxes_shard: np.ndarray):
    """Host-side packing of the 5x5 kernel + box indicator rows.
    cst [84, 400+128+84+84] f16:
      [:, 0:400]    banded conv matrices Kc_i[pw, i*80+w] = k[i, pw-w]
      [0, 400:528]  ones row for the K=1 broadcast matmul
      [0:64, 528:612]  Cm[n, p] = col interval indicator at mapped coord
      [0:64, 612:696]  Rv[n, p] = row interval indicator * validity
    Indicators reproduce the reference's exact f32 trunc/clamp box
    rasterization semantics (computed in f32, thresholds as ints)."""
    k = ker.reshape(KS, KS).astype(np.float32)
    cst = np.zeros((PAD, CST_COLS), dtype=np.float16)
    w = np.arange(W)
    for i in range(KS):
        for j in range(KS):
            cst[w + j, i * W + w] = np.float16(k[i, j])
    cst[0, KS * W : KS * W + 128] = np.float16(1.0)

    b = boxes_shard.reshape(NB2, 4).astype(np.float32)
    xc, yc, bw, bh = b[:, 0], b[:, 1], b[:, 2], b[:, 3]
    Wf = np.float32(W)
    half = np.float32(0.5)
    x1 = np.maximum(np.float32(0.0), np.trunc((xc - bw * half) * Wf)).astype(np.int32)
    y1 = np.maximum(np.float32(0.0), np.trunc((yc - bh * half) * Wf)).astype(np.int32)
    x2 = np.minimum(np.float32(W - 1), np.trunc((xc + bw * half) * Wf)).astype(np.int32)
    y2 = np.minimum(np.float32(W - 1), np.trunc((yc + bh * half) * Wf)).astype(np.int32)
    valid = (x2 > x1) & (y2 > y1)

    p = np.arange(PAD, dtype=np.float32)
    mapped = np.minimum(np.abs(p - 2.0), 158.0 - (p - 2.0)).astype(np.int32)  # [84]
    cmv = (mapped[None, :] >= x1[:, None]) & (mapped[None, :] <= x2[:, None])
    rvv = ((mapped[None, :] >= y1[:, None]) & (mapped[None, :] <= y2[:, None])
           & valid[:, None])
    cst[0:NB2, KS * W + 128 : KS * W + 128 + PAD] = cmv.astype(np.float16)
    cst[0:NB2, KS * W + 128 + PAD : CST_COLS] = rvv.astype(np.float16)
    return cst


_NC_CACHE = None


def _get_nc():
    global _NC_CACHE
    if _NC_CACHE is None:
        _NC_CACHE = _build_nc()
    return _NC_CACHE


def _run(inputs, trace=False, **kw):
    x = np.ascontiguousarray(np.asarray(inputs["x"], dtype=np.float32))
    boxes = np.ascontiguousarray(np.asarray(inputs["boxes"], dtype=np.float32))
    ker = np.ascontiguousarray(np.asarray(inputs["kernel"], dtype=np.float32))
    assert x.shape == (B, C, H, W) and boxes.shape == (B, NBOX, 4)

    nc = _get_nc()
    in_maps = []
    for i in range(N_CORES):
        bsh = boxes[i * B_LOC : (i + 1) * B_LOC]
        cst = _host_consts(ker, bsh)
        in_maps.append(
            {
                "x": x[i * B_LOC : (i + 1) * B_LOC],
                "boxes": bsh,
                "kernel": ker,
                "cst": cst,
            }
        )
    res = run_bass_kernel_spmd(nc, in_maps, core_ids=list(range(N_CORES)),
                               trace=trace, **kw)
    out = np.concatenate([r["out"] for r in res.results], axis=0)
    return out, res


def kernel(**inputs) -> np.ndarray:
    out, _ = _run(inputs, trace=False)
    return out


# revision 23
# speedup vs baseline: 1.0088x; 1.0088x over previous
"""AssistedExcitation distributed Bass kernel for 8 TRN2 NeuronCores.

Reference computation (per batch b):
    mask[h,w]  = union over 32 boxes of axis-aligned rectangles (rasterized
                 from normalized xywh boxes, trunc + clamp semantics)
    att        = 5x5 conv of reflect-padded mask with the given kernel
    out        = x + att * x        (att broadcast over 256 channels)

Sharding: pure data parallel — batch 16 is split 2-per-core across 8 cores.
No collectives needed.

Per-core algorithm (all bulk work on-device):
  * Box rasterization is a matmul: 0/1 interval-indicator rows
    Cm[n,pw] (cols) and Rv[n,ph] (rows x validity), evaluated at
    reflect-mapped padded coordinates m[p]=min(|p-2|,158-(p-2)), give
    PT[pw,ph] = #boxes covering the padded pixel via lhsT=Cm, rhs=Rv;
    binarize (>0) yields the *reflect-padded transposed* mask in one
    shot.  Cm/Rv are computed host-side in exact f32/trunc/clamp
    reference arithmetic and shipped inside the merged const tensor
    (64x84 f16 each) — the device attention path starts at the PT
    matmul the moment the consts land.
  * The 5x5 conv is 5 PSUM-accumulated matmuls with banded matrices
    Kc_i[pw,w] = k[i, pw-w]:  att[h,w] = sum_i sum_pw PT[pw,h+i]*Kc_i[pw,w].
    Kc (a pure repacking of the 25 kernel weights) and the broadcast
    ones-vector are precomputed host-side in the same const tensor.
  * (1+att) is broadcast across the 128 partitions with K=1 fp16 matmuls
    (lhsT = ones[1,128], rhs = fp16 flattened (1+att) row), evicted to
    SBUF f16, then out = x16 * att_bc on the VectorEngine (f32 result),
    streamed in [128, 1600] chunks.

Scheduling notes (the kernel is DMA-roofline-bound: ~26.4 MB of HBM
traffic per core against the ~435 GB/s 16-SDMA-engine pool; measured
~411 GB/s average with a zero-idle in-ring, within ~1% of the
achievable floor):
  * Two bulk DMA rings: the x in-stream rides the gpsimd SWDGE ring
    (casting f32 -> f16 inline), and the out-stream rides the sync
    HWDGE ring behind the tiny merged const DMA and the two att1->flat
    flatten DMAs.  Rings drain concurrently (packet round-robin at the
    SDMA engines), so the in-flood needs no trigger-order pins and
    never queues behind an out-trigger's mult semaphore.
  * With the in-stream off the HWDGE lanes entirely, the flat/const
    completions are observed on nearly-empty DMAHW lanes -- promptly,
    with no tick-order pins needed.
  * All 16 in-chunks are prefetched into 16 fp16 pool bufs up-front, so
    the in-ring streams unconditionally, never blocked on pool-slot
    reuse: during the attention-path latency the in-ring alone consumes
    the full SDMA bandwidth, making the out-start latency nearly free.
  * Chunks are k-major per batch and each batch's mults are emitted
    right after the broadcast eviction that completes their att_bc
    column range, so out0 fires ~5us after the flat lands rather than
    after the whole broadcast.
  * fp16 x-stream rel-err ~2e-4, far inside the 2e-2 gate; it halves
    SBUF footprint (enables the 16-deep in pool) and SBUF-side DMA
    writes.
"""

import numpy as np

import concourse.bass as bass
import concourse.tile as tile
from concourse import bacc, mybir
from concourse.tile_rust import add_dep_helper
from concourse.bass_utils import run_bass_kernel_spmd

F32 = mybir.dt.float32
F16 = mybir.dt.float16
ALU = mybir.AluOpType
ACT = mybir.ActivationFunctionType

N_CORES = 8
B, C, H, W, NBOX = 16, 256, 80, 80, 32
B_LOC = B // N_CORES          # 2 batches per core
HW = H * W                    # 6400
PAD = 84                      # 80 + 2*2 reflect pad
KS = 5
CH = 1600                     # free-dim chunk of the x stream
N_CHUNK = HW // CH            # 4
BC_CH = 512                   # psum bank width for the broadcast matmul

NB2 = B_LOC * NBOX            # 64 boxes across the two local batches
CST_COLS = KS * W + 128 + 2 * PAD   # kc | ones | cm | rv


def _build_nc():
    nc = bacc.Bacc(None, target_bir_lowering=False)

    x_d = nc.declare_dram_parameter("x", [B_LOC, C, H, W], F32, isOutput=False)
    nc.declare_dram_parameter("boxes", [B_LOC, NBOX, 4], F32, isOutput=False)
    nc.declare_dram_parameter("kernel", [1, 1, KS, KS], F32, isOutput=False)
    cst_d = nc.declare_dram_parameter("cst", [PAD, CST_COLS], F16, isOutput=False)
    out_d = nc.declare_dram_parameter("out", [B_LOC, C, H, W], F32, isOutput=True)

    xr = x_d.rearrange("b c h w -> b c (h w)")
    outr = out_d.rearrange("b c h w -> b c (h w)")

    with tile.TileContext(nc) as tc:
        with (
            tc.tile_pool(name="const", bufs=1) as cp,
            tc.tile_pool(name="batch", bufs=2) as bp,
            tc.tile_pool(name="attbc", bufs=2) as ap_,
            tc.tile_pool(name="xin", bufs=16) as xp,
            tc.tile_pool(name="xin0", bufs=1) as xp0,
            tc.tile_pool(name="xout", bufs=8) as op_,
            tc.tile_pool(name="ps_small", bufs=2, space=bass.MemorySpace.PSUM) as psm,
            tc.tile_pool(name="ps_bc", bufs=4, space=bass.MemorySpace.PSUM) as pbc,
        ):
            # Trigger chains: pin the issue order within the sync ring
            # (cst, flats, out0..out15) and the gpsimd ring (in0..in15).
            sync_chain = []

            def _chain_sync(bi):
                if sync_chain:
                    add_dep_helper(bi.ins, sync_chain[-1].ins, sync=False,
                                   reason="pin sync trigger order")
                sync_chain.append(bi)

            in_chain = []

            def _chain_in(bi):
                if in_chain:
                    add_dep_helper(bi.ins, in_chain[-1].ins, sync=False,
                                   reason="pin gpsimd in-trigger order")
                in_chain.append(bi)

            # Chunk 0 rides the sync HWDGE ring at the very head (f32, no
            # cast): its bytes flow ~0.8us before the gpsimd SWDGE ring can
            # start, buying streaming time the preamble otherwise wastes.
            xt0_32 = xp0.tile([128, CH], F32, name="xt0_32")
            _chain_sync(nc.sync.dma_start(xt0_32[:], xr[0, 0:128, 0:CH]))

            # Merged const DMA: next on the sync ring. Contents: banded conv
            # matrices, ones row, host-precomputed box indicator rows.
            cst = cp.tile([PAD, CST_COLS], F16)
            _chain_sync(nc.sync.dma_start(cst[:], cst_d[:]))
            kc = cst[:, 0 : KS * W]
            ones16 = cst[0:1, KS * W : KS * W + 128]
            cm = cst[0:NB2, KS * W + 128 : KS * W + 128 + PAD]
            rv = cst[0:NB2, KS * W + 128 + PAD : CST_COLS]

            # ------------- per-batch attention path (through flat) -------------
            # Both batches' rasterize+conv+flatten run first; the flats ride
            # the sync ring (empty until out0, so they drain immediately).
            flats = []
            for b in range(B_LOC):
                # rasterize: PT[pw, ph] = #boxes covering the (padded) pixel
                pt_ps = psm.tile([PAD, PAD], F32, tag="pt_ps")
                nc.tensor.matmul(
                    pt_ps[:],
                    cm[b * NBOX : (b + 1) * NBOX, :],
                    rv[b * NBOX : (b + 1) * NBOX, :],
                    start=True, stop=True,
                )
                ptm = bp.tile([PAD, PAD], F16)
                nc.vector.tensor_scalar(ptm[:], pt_ps[:], 0.5, None, op0=ALU.is_ge)

                # 5x5 conv: 5 accumulated matmuls
                att_ps = psm.tile([H, W], F32, tag="att_ps")
                for i in range(KS):
                    nc.tensor.matmul(
                        att_ps[:],
                        ptm[:, i : i + H],
                        kc[:, i * W : (i + 1) * W],
                        start=(i == 0),
                        stop=(i == KS - 1),
                    )
                # (1 + att), cast to fp16 for the cheap broadcast matmul
                att1 = bp.tile([H, W], F16)
                nc.scalar.activation(att1[:], att_ps[:], ACT.Copy, bias=1.0)
                flat = bp.tile([1, HW], F16)
                _chain_sync(nc.sync.dma_start(flat[:], att1[:]))
                flats.append(flat)

            # ---------------- main stream: out = x * (1 + att) ----------------
            # Chunks are k-major per batch so consecutive mults need
            # consecutive att_bc column ranges.  Per batch the broadcast
            # evictions (alternating vector/scalar) are interleaved with the
            # mults: mult(k) is emitted right after the eviction that
            # completes its [k*CH, (k+1)*CH) range, so out0 fires ~5us after
            # the flat lands instead of after the whole broadcast.  All 16
            # in-loads are prefetched up-front into 16 pool bufs (fp16 tiles)
            # so the in-ring streams unconditionally, never slot-blocked.
            chunks = [
                (b, chalf * 128, k * CH)
                for b in range(B_LOC)
                for k in range(N_CHUNK)
                for chalf in range(C // 128)
            ]
            xts = {}

            def _load(i):
                b, c0, o0 = chunks[i]
                xt = xp.tile([128, CH], F16, name=f"xt{i}", tag="xt")
                # SWDGE in-DMA with inline f32 -> f16 downcast
                bi = nc.gpsimd.dma_start(xt[:], xr[b, c0 : c0 + 128, o0 : o0 + CH])
                _chain_in(bi)
                xts[i] = xt

            # chunk 0 already arrived f32 on the sync ring; downcast on the
            # (idle) vector engine so the mult path stays uniform f16.
            xt0 = xp.tile([128, CH], F16, name="xt0", tag="xt")
            nc.vector.tensor_copy(xt0[:], xt0_32[:])
            xts[0] = xt0
            for i in range(1, len(chunks)):
                _load(i)

            n_per_b = len(chunks) // B_LOC
            n_ev = (HW + BC_CH - 1) // BC_CH          # 13 evictions per batch

            def _mult_out(i):
                b, c0, o0 = chunks[i]
                xt = xts.pop(i)
                ot = op_.tile([128, CH], F32, name=f"ot{i}", tag="ot")
                nc.vector.tensor_mul(ot[:], xt[:], att_bcs[b][:, o0 : o0 + CH])
                _chain_sync(
                    nc.sync.dma_start(outr[b, c0 : c0 + 128, o0 : o0 + CH], ot[:])
                )

            att_bcs = []
            for b in range(B_LOC):
                att_bc = ap_.tile([128, HW], F16, tag="att_bc")
                att_bcs.append(att_bc)
                k_next = 0
                for ci in range(n_ev):
                    off = ci * BC_CH
                    cw = min(BC_CH, HW - off)
                    bc_ps = pbc.tile([128, BC_CH], F32, tag="bc_ps")
                    nc.tensor.matmul(
                        bc_ps[:, 0:cw], ones16[:], flats[b][:, off : off + cw],
                        start=True, stop=True,
                    )
                    if ci % 2 == 1:
                        nc.vector.tensor_copy(att_bc[:, off : off + cw], bc_ps[:, 0:cw])
                    else:
                        nc.scalar.copy(att_bc[:, off : off + cw], bc_ps[:, 0:cw])
                    # emit the mults whose column range is now fully evicted
                    while k_next < N_CHUNK and (k_next + 1) * CH <= off + cw:
                        for chalf in range(C // 128):
                            _mult_out(b * n_per_b + k_next * (C // 128) + chalf)
                        k_next += 1

    if not nc.is_finalized():
        nc.finalize()
    return nc


def _host_consts(ker: np.ndarray, boxes_shard: np.ndarray):
    """Host-side packing of the 5x5 kernel + box indicator rows.
    cst [84, 400+128+84+84] f16:
      [:, 0:400]    banded conv matrices Kc_i[pw, i*80+w] = k[i, pw-w]
      [0, 400:528]  ones row for the K=1 broadcast matmul
      [0:64, 528:612]  Cm[n, p] = col interval indicator at mapped coord
      [0:64, 612:696]  Rv[n, p] = row interval indicator * validity
    Indicators reproduce the reference's exact f32 trunc/clamp box
    rasterization semantics (computed in f32, thresholds as ints)."""
    k = ker.reshape(KS, KS).astype(np.float32)
    cst = np.zeros((PAD, CST_COLS), dtype=np.float16)
    w = np.arange(W)
    for i in range(KS):
        for j in range(KS):
            cst[w + j, i * W + w] = np.float16(k[i, j])
    cst[0, KS * W : KS * W + 128] = np.float16(1.0)

    b = boxes_shard.reshape(NB2, 4).astype(np.float32)
    xc, yc, bw, bh = b[:, 0], b[:, 1], b[:, 2], b[:, 3]
    Wf = np.float32(W)
    half = np.float32(0.5)
    x1 = np.maximum(np.float32(0.0), np.trunc((xc - bw * half) * Wf)).astype(np.int32)
    y1 = np.maximum(np.float32(0.0), np.trunc((yc - bh * half) * Wf)).astype(np.int32)
    x2 = np.minimum(np.float32(W - 1), np.trunc((xc + bw * half) * Wf)).astype(np.int32)
    y2 = np.minimum(np.float32(W - 1), np.trunc((yc + bh * half) * Wf)).astype(np.int32)
    valid = (x2 > x1) & (y2 > y1)

    p = np.arange(PAD, dtype=np.float32)
    mapped = np.minimum(np.abs(p - 2.0), 158.0 - (p - 2.0)).astype(np.int32)  # [84]
    cmv = (mapped[None, :] >= x1[:, None]) & (mapped[None, :] <= x2[:, None])
    rvv = ((mapped[None, :] >= y1[:, None]) & (mapped[None, :] <= y2[:, None])
           & valid[:, None])
    cst[0:NB2, KS * W + 128 : KS * W + 128 + PAD] = cmv.astype(np.float16)
    cst[0:NB2, KS * W + 128 + PAD : CST_COLS] = rvv.astype(np.float16)
    return cst


_NC_CACHE = None


def _get_nc():
    global _NC_CACHE
    if _NC_CACHE is None:
        _NC_CACHE = _build_nc()
    return _NC_CACHE


def _run(inputs, trace=False, **kw):
    x = np.ascontiguousarray(np.asarray(inputs["x"], dtype=np.float32))
    boxes = np.ascontiguousarray(np.asarray(inputs["boxes"], dtype=np.float32))
    ker = np.ascontiguousarray(np.asarray(inputs["kernel"], dtype=np.float32))
    assert x.shape == (B, C, H, W) and boxes.shape == (B, NBOX, 4)

    nc = _get_nc()
    in_maps = []
    for i in range(N_CORES):
        bsh = boxes[i * B_LOC : (i + 1) * B_LOC]
        cst = _host_consts(ker, bsh)
        in_maps.append(
            {
                "x": x[i * B_LOC : (i + 1) * B_LOC],
                "boxes": bsh,
                "kernel": ker,
                "cst": cst,
            }
        )
    res = run_bass_kernel_spmd(nc, in_maps, core_ids=list(range(N_CORES)),
                               trace=trace, **kw)
    out = np.concatenate([r["out"] for r in res.results], axis=0)
    return out, res


def kernel(**inputs) -> np.ndarray:
    out, _ = _run(inputs, trace=False)
    return out


# revision 24
# speedup vs baseline: 1.0110x; 1.0022x over previous
"""AssistedExcitation distributed Bass kernel for 8 TRN2 NeuronCores.

Reference computation (per batch b):
    mask[h,w]  = union over 32 boxes of axis-aligned rectangles (rasterized
                 from normalized xywh boxes, trunc + clamp semantics)
    att        = 5x5 conv of reflect-padded mask with the given kernel
    out        = x + att * x        (att broadcast over 256 channels)

Sharding: pure data parallel — batch 16 is split 2-per-core across 8 cores.
No collectives needed.

Per-core algorithm (all bulk work on-device):
  * Box rasterization is a matmul: 0/1 interval-indicator rows
    Cm[n,pw] (cols) and Rv[n,ph] (rows x validity), evaluated at
    reflect-mapped padded coordinates m[p]=min(|p-2|,158-(p-2)), give
    PT[pw,ph] = #boxes covering the padded pixel via lhsT=Cm, rhs=Rv;
    binarize (>0) yields the *reflect-padded transposed* mask in one
    shot.  Cm/Rv are computed host-side in exact f32/trunc/clamp
    reference arithmetic and shipped inside the merged const tensor
    (64x84 f16 each) — the device attention path starts at the PT
    matmul the moment the consts land.
  * The 5x5 conv is 5 PSUM-accumulated matmuls with banded matrices
    Kc_i[pw,w] = k[i, pw-w]:  att[h,w] = sum_i sum_pw PT[pw,h+i]*Kc_i[pw,w].
    Kc (a pure repacking of the 25 kernel weights) and the broadcast
    ones-vector are precomputed host-side in the same const tensor.
  * (1+att) is broadcast across the 128 partitions with K=1 fp16 matmuls
    (lhsT = ones[1,128], rhs = fp16 flattened (1+att) row), evicted to
    SBUF f16, then out = x16 * att_bc on the VectorEngine (f32 result),
    streamed in [128, 1600] chunks.

Scheduling notes (the kernel is DMA-roofline-bound: ~26.4 MB of HBM
traffic per core against the ~435 GB/s 16-SDMA-engine pool; measured
~411 GB/s average with a zero-idle in-ring, within ~1% of the
achievable floor):
  * Two bulk DMA rings: the x in-stream rides the gpsimd SWDGE ring
    (casting f32 -> f16 inline), and the out-stream rides the sync
    HWDGE ring behind chunk 0 (pulled f32 via HWDGE at the ring head --
    its bytes flow ~0.8us before the SWDGE ring can start), the tiny
    merged const DMA, and the two att1->flat flatten DMAs.  Rings drain
    concurrently (packet round-robin at the SDMA engines), so the
    in-flood needs no trigger-order pins and never queues behind an
    out-trigger's mult semaphore.
  * With the in-stream off the HWDGE lanes entirely, the flat/const
    completions are observed on nearly-empty DMAHW lanes -- promptly,
    with no tick-order pins needed.
  * All 16 in-chunks are prefetched into 16 fp16 pool bufs up-front, so
    the in-ring streams unconditionally, never blocked on pool-slot
    reuse: during the attention-path latency the in-ring alone consumes
    the full SDMA bandwidth, making the out-start latency nearly free.
  * Chunks are k-major per batch and each batch's mults are emitted
    right after the broadcast eviction that completes their att_bc
    column range, so out0 fires ~5us after the flat lands rather than
    after the whole broadcast.
  * fp16 x-stream rel-err ~2e-4, far inside the 2e-2 gate; it halves
    SBUF footprint (enables the 16-deep in pool) and SBUF-side DMA
    writes.
"""

import numpy as np

import concourse.bass as bass
import concourse.tile as tile
from concourse import bacc, mybir
from concourse.tile_rust import add_dep_helper
from concourse.bass_utils import run_bass_kernel_spmd

F32 = mybir.dt.float32
F16 = mybir.dt.float16
ALU = mybir.AluOpType
ACT = mybir.ActivationFunctionType

N_CORES = 8
B, C, H, W, NBOX = 16, 256, 80, 80, 32
B_LOC = B // N_CORES          # 2 batches per core
HW = H * W                    # 6400
PAD = 84                      # 80 + 2*2 reflect pad
KS = 5
CH = 1600                     # free-dim chunk of the x stream
N_CHUNK = HW // CH            # 4
BC_CH = 512                   # psum bank width for the broadcast matmul

NB2 = B_LOC * NBOX            # 64 boxes across the two local batches
CST_COLS = KS * W + 128 + 2 * PAD   # kc | ones | cm | rv


def _build_nc():
    nc = bacc.Bacc(None, target_bir_lowering=False)

    x_d = nc.declare_dram_parameter("x", [B_LOC, C, H, W], F32, isOutput=False)
    nc.declare_dram_parameter("boxes", [B_LOC, NBOX, 4], F32, isOutput=False)
    nc.declare_dram_parameter("kernel", [1, 1, KS, KS], F32, isOutput=False)
    cst_d = nc.declare_dram_parameter("cst", [PAD, CST_COLS], F16, isOutput=False)
    out_d = nc.declare_dram_parameter("out", [B_LOC, C, H, W], F32, isOutput=True)

    xr = x_d.rearrange("b c h w -> b c (h w)")
    outr = out_d.rearrange("b c h w -> b c (h w)")

    with tile.TileContext(nc) as tc:
        with (
            tc.tile_pool(name="const", bufs=1) as cp,
            tc.tile_pool(name="batch", bufs=2) as bp,
            tc.tile_pool(name="attbc", bufs=2) as ap_,
            tc.tile_pool(name="xin", bufs=16) as xp,
            tc.tile_pool(name="xin0", bufs=1) as xp0,
            tc.tile_pool(name="xout", bufs=8) as op_,
            tc.tile_pool(name="ps_small", bufs=2, space=bass.MemorySpace.PSUM) as psm,
            tc.tile_pool(name="ps_bc", bufs=4, space=bass.MemorySpace.PSUM) as pbc,
        ):
            # Trigger chains: pin the issue order within the sync ring
            # (cst, flats, out0..out15) and the gpsimd ring (in0..in15).
            sync_chain = []

            def _chain_sync(bi):
                if sync_chain:
                    add_dep_helper(bi.ins, sync_chain[-1].ins, sync=False,
                                   reason="pin sync trigger order")
                sync_chain.append(bi)

            in_chain = []

            def _chain_in(bi):
                if in_chain:
                    add_dep_helper(bi.ins, in_chain[-1].ins, sync=False,
                                   reason="pin gpsimd in-trigger order")
                in_chain.append(bi)

            # Chunk 0 rides the sync HWDGE ring at the very head (f32, no
            # cast): its bytes flow ~0.8us before the gpsimd SWDGE ring can
            # start, buying streaming time the preamble otherwise wastes.
            xt0_32 = xp0.tile([128, CH], F32, name="xt0_32")
            _chain_sync(nc.sync.dma_start(xt0_32[:], xr[0, 0:128, 0:CH]))

            # Merged const DMA: next on the sync ring. Contents: banded conv
            # matrices, ones row, host-precomputed box indicator rows.
            cst = cp.tile([PAD, CST_COLS], F16)
            _chain_sync(nc.sync.dma_start(cst[:], cst_d[:]))
            kc = cst[:, 0 : KS * W]
            ones16 = cst[0:1, KS * W : KS * W + 128]
            cm = cst[0:NB2, KS * W + 128 : KS * W + 128 + PAD]
            rv = cst[0:NB2, KS * W + 128 + PAD : CST_COLS]

            # ------------- per-batch attention path (through flat) -------------
            # Both batches' rasterize+conv+flatten run first; the flats ride
            # the sync ring (empty until out0, so they drain immediately).
            flats = []
            for b in range(B_LOC):
                # rasterize: PT[pw, ph] = #boxes covering the (padded) pixel
                pt_ps = psm.tile([PAD, PAD], F32, tag="pt_ps")
                nc.tensor.matmul(
                    pt_ps[:],
                    cm[b * NBOX : (b + 1) * NBOX, :],
                    rv[b * NBOX : (b + 1) * NBOX, :],
                    start=True, stop=True,
                )
                ptm = bp.tile([PAD, PAD], F16)
                nc.vector.tensor_scalar(ptm[:], pt_ps[:], 0.5, None, op0=ALU.is_ge)

                # 5x5 conv: 5 accumulated matmuls
                att_ps = psm.tile([H, W], F32, tag="att_ps")
                for i in range(KS):
                    nc.tensor.matmul(
                        att_ps[:],
                        ptm[:, i : i + H],
                        kc[:, i * W : (i + 1) * W],
                        start=(i == 0),
                        stop=(i == KS - 1),
                    )
                # (1 + att), cast to fp16 for the cheap broadcast matmul
                att1 = bp.tile([H, W], F16)
                nc.scalar.activation(att1[:], att_ps[:], ACT.Copy, bias=1.0)
                flat = bp.tile([1, HW], F16)
                _chain_sync(nc.sync.dma_start(flat[:], att1[:]))
                flats.append(flat)

            # ---------------- main stream: out = x * (1 + att) ----------------
            # Chunks are k-major per batch so consecutive mults need
            # consecutive att_bc column ranges.  Per batch the broadcast
            # evictions (alternating vector/scalar) are interleaved with the
            # mults: mult(k) is emitted right after the eviction that
            # completes its [k*CH, (k+1)*CH) range, so out0 fires ~5us after
            # the flat lands instead of after the whole broadcast.  All 16
            # in-loads are prefetched up-front into 16 pool bufs (fp16 tiles)
            # so the in-ring streams unconditionally, never slot-blocked.
            chunks = [
                (b, chalf * 128, k * CH)
                for b in range(B_LOC)
                for k in range(N_CHUNK)
                for chalf in range(C // 128)
            ]
            xts = {}

            def _load(i):
                b, c0, o0 = chunks[i]
                xt = xp.tile([128, CH], F16, name=f"xt{i}", tag="xt")
                # SWDGE in-DMA with inline f32 -> f16 downcast
                bi = nc.gpsimd.dma_start(xt[:], xr[b, c0 : c0 + 128, o0 : o0 + CH])
                _chain_in(bi)
                xts[i] = xt

            # chunk 0 already arrived f32 on the sync ring; downcast on the
            # (idle) vector engine so the mult path stays uniform f16.
            xt0 = xp.tile([128, CH], F16, name="xt0", tag="xt")
            nc.vector.tensor_copy(xt0[:], xt0_32[:])
            xts[0] = xt0
            for i in range(1, len(chunks)):
                _load(i)

            n_per_b = len(chunks) // B_LOC
            n_ev = (HW + BC_CH - 1) // BC_CH          # 13 evictions per batch

            def _mult_out(i):
                b, c0, o0 = chunks[i]
                xt = xts.pop(i)
                ot = op_.tile([128, CH], F32, name=f"ot{i}", tag="ot")
                nc.vector.tensor_mul(ot[:], xt[:], att_bcs[b][:, o0 : o0 + CH])
                _chain_sync(
                    nc.sync.dma_start(outr[b, c0 : c0 + 128, o0 : o0 + CH], ot[:])
                )

            att_bcs = []
            for b in range(B_LOC):
                att_bc = ap_.tile([128, HW], F16, tag="att_bc")
                att_bcs.append(att_bc)
                k_next = 0
                for ci in range(n_ev):
                    off = ci * BC_CH
                    cw = min(BC_CH, HW - off)
                    bc_ps = pbc.tile([128, BC_CH], F32, tag="bc_ps")
                    nc.tensor.matmul(
                        bc_ps[:, 0:cw], ones16[:], flats[b][:, off : off + cw],
                        start=True, stop=True,
                    )
                    if ci % 2 == 1:
                        nc.vector.tensor_copy(att_bc[:, off : off + cw], bc_ps[:, 0:cw])
                    else:
                        nc.scalar.copy(att_bc[:, off : off + cw], bc_ps[:, 0:cw])
                    # emit the mults whose column range is now fully evicted
                    while k_next < N_CHUNK and (k_next + 1) * CH <= off + cw:
                        for chalf in range(C // 128):
                            _mult_out(b * n_per_b + k_next * (C // 128) + chalf)
                        k_next += 1

    if not nc.is_finalized():
        nc.finalize()
    return nc


def _host_consts(ker: np.ndarray, boxes_shard: np.ndarray):
    """Host-side packing of the 5x5 kernel + box indicator rows.
    cst [84, 400+128+84+84] f16:
      [:, 0:400]    banded conv matrices Kc_i[pw, i*80+w] = k[i, pw-w]
      [0, 400:528]  ones row for the K=1 broadcast matmul
      [0:64, 528:612]  Cm[n, p] = col interval indicator at mapped coord
      [0:64, 612:696]  Rv[n, p] = row interval indicator * validity
    Indicators reproduce the reference's exact f32 trunc/clamp box
    rasterization semantics (computed in f32, thresholds as ints)."""
    k = ker.reshape(KS, KS).astype(np.float32)
    cst = np.zeros((PAD, CST_COLS), dtype=np.float16)
    w = np.arange(W)
    for i in range(KS):
        for j in range(KS):
            cst[w + j, i * W + w] = np.float16(k[i, j])
    cst[0, KS * W : KS * W + 128] = np.float16(1.0)

    b = boxes_shard.reshape(NB2, 4).astype(np.float32)
    xc, yc, bw, bh = b[:, 0], b[:, 1], b[:, 2], b[:, 3]
    Wf = np.float32(W)
    half = np.float32(0.5)
    x1 = np.maximum(np.float32(0.0), np.trunc((xc - bw * half) * Wf)).astype(np.int32)
    y1 = np.maximum(np.float32(0.0), np.trunc((yc - bh * half) * Wf)).astype(np.int32)
    x2 = np.minimum(np.float32(W - 1), np.trunc((xc + bw * half) * Wf)).astype(np.int32)
    y2 = np.minimum(np.float32(W - 1), np.trunc((yc + bh * half) * Wf)).astype(np.int32)
    valid = (x2 > x1) & (y2 > y1)

    p = np.arange(PAD, dtype=np.float32)
    mapped = np.minimum(np.abs(p - 2.0), 158.0 - (p - 2.0)).astype(np.int32)  # [84]
    cmv = (mapped[None, :] >= x1[:, None]) & (mapped[None, :] <= x2[:, None])
    rvv = ((mapped[None, :] >= y1[:, None]) & (mapped[None, :] <= y2[:, None])
           & valid[:, None])
    cst[0:NB2, KS * W + 128 : KS * W + 128 + PAD] = cmv.astype(np.float16)
    cst[0:NB2, KS * W + 128 + PAD : CST_COLS] = rvv.astype(np.float16)
    return cst


_NC_CACHE = None


def _get_nc():
    global _NC_CACHE
    if _NC_CACHE is None:
        _NC_CACHE = _build_nc()
    return _NC_CACHE


def _run(inputs, trace=False, **kw):
    x = np.ascontiguousarray(np.asarray(inputs["x"], dtype=np.float32))
    boxes = np.ascontiguousarray(np.asarray(inputs["boxes"], dtype=np.float32))
    ker = np.ascontiguousarray(np.asarray(inputs["kernel"], dtype=np.float32))
    assert x.shape == (B, C, H, W) and boxes.shape == (B, NBOX, 4)

    nc = _get_nc()
    in_maps = []
    for i in range(N_CORES):
        bsh = boxes[i * B_LOC : (i + 1) * B_LOC]
        cst = _host_consts(ker, bsh)
        in_maps.append(
            {
                "x": x[i * B_LOC : (i + 1) * B_LOC],
                "boxes": bsh,
                "kernel": ker,
                "cst": cst,
            }
        )
    res = run_bass_kernel_spmd(nc, in_maps, core_ids=list(range(N_CORES)),
                               trace=trace, **kw)
    out = np.concatenate([r["out"] for r in res.results], axis=0)
    return out, res


def kernel(**inputs) -> np.ndarray:
    out, _ = _run(inputs, trace=False)
    return out
